# revision 20
# baseline (speedup 1.0000x reference)
"""MixAdapter: alpha-weighted adapter superposition + joint layernorm + bottleneck MLP.

Two SPMD launches on 8 NeuronCores (HW-calibrated engine assignment):

  Launch A ("merge"): fp16 adapter stacks sharded across cores (~5MB each).
    Scaled copies alpha_n*W_n run on ACT (20) and DVE (5); two parallel
    accumulation chains run on DVE (tensor_tensor, 2x mode) and gpsimd,
    combined at the end.  Host gathers the 0.8MB of merged params.

  Host folding (tiny): wdTw = W_ln*W_down scaled+quantized to fp8e4,
    wuT zero-padded/scaled/quantized, P/Q bias vectors.

  Launch B ("main"): batch elem k -> core k.
    - x^T fp16 in; ACT downcasts all 8 d-chunks to fp8 (x*32) with accum_out
      providing S1 (the sum).
    - S2: DVE squares x pairwise (tensor_tensor, 2x); PE ones-matmuls
      column-sum the squares into a PSUM accumulator; one small DVE reduce.
    - Down/up projections: fp8e4 DoubleRow matmuls with 1024-wide moving
      APs (512 output columns per instruction).
    - ReLU on ACT folds rstd/bias, requantizes h to fp8.
    - Residual y = psum/(WU*H) + x: dt 0-5 DVE stt from PSUM; dt 6-7 ACT
      scaled-evict + gpsimd add.  y written fp16, host upcasts.
"""

import numpy as np
import ml_dtypes

from concourse import bacc, mybir, tile
import concourse.bass as bass
from concourse.bass_utils import run_bass_kernel_spmd

B, S, D, BOT, N = 8, 2048, 1024, 400, 25
NCORES = 8
EPS = 1e-5
FP32 = mybir.dt.float32
F16 = mybir.dt.float16
F8 = mybir.dt.float8e4
U8 = mybir.dt.uint8
NP_F8 = ml_dtypes.float8_e4m3
F8_MAX = 240.0

DC = D // 128        # 8 d-chunks
OC = 4               # o-chunks (400 -> 3x128 + 16; padded to 512 for up-proj)
O_SZ = [128, 128, 128, 16]
NSBP = S // 512      # 4 psum-bank-wide moving groups

X_SCL = 32.0
W_SCL = 4096.0
WU_SCL = 1024.0
H_SCL = 64.0
PSD_INV = 1.0 / (W_SCL * X_SCL)
PSU_INV = 1.0 / (WU_SCL * H_SCL)

USE_F32R = False  # kept for test.py compatibility

WD_ROWS = BOT // NCORES
WU_ROWS = D // NCORES
MF = 400 + 400 + 2 * DC

DR = mybir.MatmulPerfMode.DoubleRow


# ---------------------------------------------------------------------------
# Launch A: alpha-weighted merge of the adapter stacks (sharded over cores)
# ---------------------------------------------------------------------------

N_ACT_COPY = 20   # adapters whose scaled copy runs on ACT (rest on DVE)
N_DVE_ACC = 16    # adapters 1..15 accumulate on DVE; 17.. on gpsimd


def build_merge_nc():
    nc = bacc.Bacc("TRN2", target_bir_lowering=False, debug=False,
                   enable_asserts=False, num_devices=NCORES)

    # adapters per stack DMA: tiny first group so the DVE chain starts early
    GRPS = [1, 4, 5, 5, 5, 5]
    stack = nc.dram_tensor("stack", [128, N * MF], F16, kind="ExternalInput")
    alphas = nc.dram_tensor("alphas", [1, N], FP32, kind="ExternalInput")
    out_m = nc.dram_tensor("out_m", [128, MF], F16, kind="ExternalOutput")

    with tile.TileContext(nc) as tc:
        with (
            tc.tile_pool(name="consts", bufs=1) as consts,
            tc.tile_pool(name="acc", bufs=1) as accp,
            tc.tile_pool(name="stk", bufs=3) as stk_pool,
            tc.tile_pool(name="psum", bufs=1, space="PSUM") as psum,
        ):
            a_sb = consts.tile([1, N], FP32)
            nc.sync.dma_start(a_sb[:], alphas[:])
            ones_row = consts.tile([1, 128], FP32)
            nc.vector.memset(ones_row[:], 1.0)
            pa = psum.tile([128, N], FP32)
            nc.tensor.matmul(pa[:], ones_row[:], a_sb[:], start=True, stop=True)
            a_bc = consts.tile([128, N], FP32)
            nc.scalar.copy(a_bc[:], pa[:])

            # single fused copy+scale+add chain on DVE (one op per adapter)
            acc = accp.tile([128, MF], F16)
            n0 = 0
            for g, grp in enumerate(GRPS):
                st = stk_pool.tile([128, grp * MF], F16, name=f"st{g}", tag="st")
                nc.sync.dma_start(st[:], stack[:, n0 * MF:(n0 + grp) * MF])
                for q in range(grp):
                    n = n0 + q
                    al = a_bc[:, n:n + 1]
                    if n == 0:
                        nc.vector.tensor_scalar_mul(acc[:], st[:, 0:MF], al)
                    else:
                        nc.vector.scalar_tensor_tensor(
                            acc[:], st[:, q * MF:(q + 1) * MF], al, acc[:],
                            mybir.AluOpType.mult, mybir.AluOpType.add)
                n0 += grp

            nc.sync.dma_start(out_m[:], acc[:])

    nc.finalize()
    return nc


# ---------------------------------------------------------------------------
# Launch B: layernorm + down/up projections, one batch element per core
# ---------------------------------------------------------------------------

def build_main_nc():
    nc = bacc.Bacc("TRN2", target_bir_lowering=False, debug=False,
                   enable_asserts=False, num_devices=NCORES)

    xT16 = nc.dram_tensor("xT16", [128, DC, S], F16, kind="ExternalInput")
    wd8 = nc.dram_tensor("wd8", [128, DC, BOT], U8, kind="ExternalInput")
    wu8 = nc.dram_tensor("wu8", [128, OC, D], U8, kind="ExternalInput")
    pq = nc.dram_tensor("pq", [128, 2 * OC], FP32, kind="ExternalInput")
    yT = nc.dram_tensor("yT", [128, NSBP, DC, 512], F16, kind="ExternalOutput")

    inv1 = 1.0 / (X_SCL * float(S * D))   # S1 -> mu
    inv2 = 1.0 / float(S * D)             # S2 -> E[x^2]

    with tile.TileContext(nc) as tc:
        with (
            tc.tile_pool(name="xt", bufs=1) as xt_pool,
            tc.tile_pool(name="x8", bufs=1) as x8_pool,
            tc.tile_pool(name="ht", bufs=1) as ht_pool,
            tc.tile_pool(name="w", bufs=1) as w_pool,
            tc.tile_pool(name="small", bufs=1) as small,
            tc.tile_pool(name="sq", bufs=3) as sq_pool,
            tc.tile_pool(name="yo", bufs=4) as yo_pool,
            tc.tile_pool(name="pmd", bufs=2, space="PSUM") as pmd,
            tc.tile_pool(name="pmu", bufs=3, space="PSUM") as pmu,
            tc.tile_pool(name="pst", bufs=1, space="PSUM") as pstp,
        ):
            # ---- x stream: 4 chunk-pair DMAs; weights interleaved ----
            xt16 = []
            for j in range(DC // 2):
                t = xt_pool.tile([128, 2, S], F16, name=f"xt{j}", tag=f"xt{j}")
                nc.sync.dma_start(t[:], xT16[:, 2 * j:2 * j + 2, :])
                xt16.append(t)

            wd_sb = w_pool.tile([128, DC, BOT], F8, tag="wd")
            nc.sync.dma_start(wd_sb[:].bitcast(U8), wd8[:])
            wu_sb = w_pool.tile([128, OC, D], F8, tag="wu")
            nc.sync.dma_start(wu_sb[:].bitcast(U8), wu8[:])
            pq_sb = small.tile([128, 2 * OC], FP32)
            nc.sync.dma_start(pq_sb[:], pq[:])

            ht = [ht_pool.tile([128, 2, S], F8, name=f"ht{j}", tag=f"ht{j}")
                  for j in range(2)]
            nc.gpsimd.memset(ht[1][:, 1, :], 0.0)

            ones16 = small.tile([128, 1], F16)
            nc.vector.memset(ones16[:], 1.0)

            # stats PSUM bank: colsums of x^2 on partition 0, scalar matmul
            # outputs parked at other partitions/columns of the same bank
            pstc = pstp.tile([128, 512], FP32)
            stat_ps = pstc[0:1, 0:512]

            # ---- downcast (ACT, S1 via accum) + squares (DVE) + colsums (PE)
            x8 = []
            sums = small.tile([128, DC], FP32)
            for j in range(DC // 2):
                t8 = x8_pool.tile([128, 2, S], F8, name=f"x8{j}", tag=f"x8{j}")
                for i in range(2):
                    c = 2 * j + i
                    if c < 5:
                        nc.scalar.activation(t8[:, i, :], xt16[j][:, i, :],
                                             mybir.ActivationFunctionType.Copy,
                                             scale=X_SCL,
                                             accum_out=sums[:, c:c + 1])
                    else:
                        nc.vector.tensor_scalar(t8[:, i, :], xt16[j][:, i, :],
                                                X_SCL, 0.0,
                                                mybir.AluOpType.mult,
                                                mybir.AluOpType.add,
                                                accum_out=sums[:, c:c + 1])
                sq = sq_pool.tile([128, 2, S], F16, name=f"sq{j}", tag="sq")
                nc.vector.tensor_tensor(sq[:], xt16[j][:], xt16[j][:],
                                        mybir.AluOpType.mult)
                for m in range(8):
                    nc.tensor.matmul(stat_ps[:],
                                     ones16[:],
                                     sq[:, m // 4, 512 * (m % 4):512 * (m % 4 + 1)],
                                     start=(j == 0 and m == 0),
                                     stop=(j == 3 and m == 7))
                x8.append(t8)

            # ---- stats scalar chain ----
            s1 = small.tile([128, 1], FP32)
            nc.vector.tensor_reduce(s1[:], sums[:], mybir.AxisListType.X,
                                    mybir.AluOpType.add)
            inv1_col = small.tile([128, 1], FP32)
            nc.vector.memset(inv1_col[:], inv1)
            ones_row = small.tile([1, 128], FP32)
            nc.vector.memset(ones_row[:], 1.0)

            nc.tensor.matmul(pstc[32:33, 0:1], inv1_col[:], s1[:],
                             start=True, stop=True)

            sc = small.tile([1, 8], FP32)
            mu, s2r, e2, nvar, std, rstd, rs, mrn = (sc[:, i:i + 1] for i in range(8))
            mr = small.tile([1, 1], FP32)
            eps_sb = small.tile([1, 1], FP32)
            nc.vector.memset(eps_sb[:], EPS)
            nc.scalar.copy(mu, pstc[32:33, 0:1])
            nc.vector.tensor_reduce(s2r, stat_ps[:], mybir.AxisListType.X,
                                    mybir.AluOpType.add)
            nc.vector.tensor_scalar_mul(e2, s2r, inv2)
            # nvar = mu^2 - e2 ; std = sqrt(-nvar + eps) ; rstd = 1/std
            nc.vector.scalar_tensor_tensor(nvar, mu, mu, e2,
                                           mybir.AluOpType.mult,
                                           mybir.AluOpType.subtract)
            nc.scalar.activation(std, nvar, mybir.ActivationFunctionType.Sqrt,
                                 bias=eps_sb[:], scale=-1.0)
            nc.vector.reciprocal(rstd, std)
            nc.vector.tensor_scalar_mul(rs, rstd, H_SCL * PSD_INV)
            nc.vector.tensor_tensor(mr, mu, rstd, mybir.AluOpType.mult)
            nc.vector.tensor_scalar_mul(mrn, mr, -H_SCL)

            nc.tensor.matmul(pstc[:, 2:4], ones_row[:], sc[:, 6:8],
                             start=True, stop=True)
            bc = small.tile([128, 2], FP32)
            nc.scalar.copy(bc[:], pstc[:, 2:4])

            bias_sb = small.tile([128, OC], FP32)
            nc.vector.scalar_tensor_tensor(
                bias_sb[:], pq_sb[:, OC:2 * OC], bc[:, 1:2], pq_sb[:, 0:OC],
                mybir.AluOpType.mult, mybir.AluOpType.add)

            # ---- down-proj (fp8 DoubleRow, 1024-wide moving) + ReLU ----
            for ot in range(OC):
                osz = O_SZ[ot]
                for sbpp in range(NSBP // 2):
                    ph = pmd.tile([128, 1024], FP32, name=f"ph{ot}_{sbpp}", tag="mmd")
                    for half in range(2):
                        sbp = 2 * sbpp + half
                        for kk in range(4):
                            nc.tensor.matmul(
                                ph[:osz, 512 * half:512 * (half + 1)],
                                wd_sb[:, 2 * kk:2 * kk + 2, 128 * ot:128 * ot + osz],
                                x8[kk][:, :, 512 * sbp:512 * (sbp + 1)],
                                start=(kk == 0), stop=(kk == 3), perf_mode=DR)
                    nc.scalar.activation(
                        ht[ot // 2][:osz, ot % 2, 1024 * sbpp:1024 * (sbpp + 1)],
                        ph[:osz, :],
                        mybir.ActivationFunctionType.Relu,
                        bias=bias_sb[:osz, ot:ot + 1], scale=bc[:osz, 0:1])

            # ---- up-proj (fp8 DoubleRow) + residual + store ----
            for sbp in range(NSBP):
                yo = yo_pool.tile([128, DC, 512], F16, name=f"yo{sbp}", tag="yo")
                for dt in range(DC):
                    pu = pmu.tile([128, 512], FP32, name=f"pu{dt}_{sbp}", tag="mmu")
                    for kk in range(2):
                        nc.tensor.matmul(
                            pu[:],
                            wu_sb[:, 2 * kk:2 * kk + 2, 128 * dt:128 * (dt + 1)],
                            ht[kk][:, :, 512 * sbp:512 * (sbp + 1)],
                            start=(kk == 0), stop=(kk == 1), perf_mode=DR)
                    xs = xt16[dt // 2][:, dt % 2, 512 * sbp:512 * (sbp + 1)]
                    if dt < 6:
                        nc.vector.scalar_tensor_tensor(
                            yo[:, dt, :], pu[:], PSU_INV, xs,
                            mybir.AluOpType.mult, mybir.AluOpType.add)
                    else:
                        nc.scalar.activation(yo[:, dt, :], pu[:],
                                             mybir.ActivationFunctionType.Copy,
                                             scale=PSU_INV)
                        nc.gpsimd.tensor_tensor(yo[:, dt, :], yo[:, dt, :], xs,
                                                mybir.AluOpType.add)
                nc.sync.dma_start(yT[:, sbp, :, :], yo[:])

    nc.finalize()
    return nc


# ---------------------------------------------------------------------------
# Host-side orchestration
# ---------------------------------------------------------------------------

def prep_merge_inputs(alphas, W_down_all, W_up_all, W_ln_all, b_ln_all):
    a_in = np.ascontiguousarray(alphas.reshape(1, N)).astype(np.float32)
    wln = W_ln_all.reshape(N, DC, 128).transpose(0, 2, 1)
    bln = b_ln_all.reshape(N, DC, 128).transpose(0, 2, 1)
    ln_blk = np.concatenate([wln, bln], axis=2)             # [N,128,16]
    in_maps = []
    for k in range(NCORES):
        wd_k = W_down_all[:, WD_ROWS * k:WD_ROWS * (k + 1), :].reshape(N, 128, 400)
        wu_k = W_up_all[:, WU_ROWS * k:WU_ROWS * (k + 1), :]
        stack = np.concatenate([wd_k, wu_k, ln_blk], axis=2).astype(np.float16)
        # all adapters side-by-side in the free dim: [128, N*MF]
        stack = stack.transpose(1, 0, 2).reshape(128, N * MF)
        in_maps.append({"stack": np.ascontiguousarray(stack), "alphas": a_in})
    return in_maps


def _to_f8(a):
    return np.clip(a, -F8_MAX, F8_MAX).astype(NP_F8)


def assemble_merge(results):
    W_down = np.concatenate(
        [results[k]["out_m"][:, 0:400].astype(np.float32).reshape(WD_ROWS, D)
         for k in range(NCORES)], axis=0)                   # [BOT, D]
    W_up = np.concatenate(
        [results[k]["out_m"][:, 400:800].astype(np.float32)
         for k in range(NCORES)], axis=0)                   # [D, BOT]
    ln = results[0]["out_m"][:, 800:].astype(np.float32)
    W_ln = ln[:, 0:DC].T.reshape(D)
    b_ln = ln[:, DC:2 * DC].T.reshape(D)

    wdT = W_down.T * (W_ln * W_SCL)[:, None]
    wd8 = _to_f8(wdT.reshape(DC, 128, BOT).transpose(1, 0, 2))

    wuT_pad = np.zeros((4 * 128, D), dtype=np.float32)
    wuT_pad[:BOT] = W_up.T * WU_SCL
    wu8 = _to_f8(wuT_pad.reshape(OC, 128, D).transpose(1, 0, 2))

    P = W_down @ b_ln
    Q = W_down @ W_ln
    pq = np.zeros((128, 2 * OC), dtype=np.float32)
    Pp = np.zeros(512, dtype=np.float32); Pp[:BOT] = H_SCL * P
    Qp = np.zeros(512, dtype=np.float32); Qp[:BOT] = Q
    pq[:, 0:OC] = Pp.reshape(OC, 128).T
    pq[:, OC:2 * OC] = Qp.reshape(OC, 128).T
    return (np.ascontiguousarray(wd8).view(np.uint8),
            np.ascontiguousarray(wu8).view(np.uint8),
            np.ascontiguousarray(pq))


def prep_main_inputs(x, wd8, wu8, pq):
    in_maps = []
    for k in range(NCORES):
        xt = x[k].T.reshape(DC, 128, S).transpose(1, 0, 2).astype(np.float16)
        in_maps.append({"xT16": np.ascontiguousarray(xt),
                        "wd8": wd8, "wu8": wu8, "pq": pq})
    return in_maps


def assemble_output(results):
    out = np.empty((B, S, D), dtype=np.float32)
    for k in range(NCORES):
        y = results[k]["yT"].astype(np.float32)   # [128, NSBP, DC, 512]
        out[k] = y.transpose(1, 3, 2, 0).reshape(S, D)
    return out


_NC_CACHE = {}


def _get_nc(which):
    if which not in _NC_CACHE:
        _NC_CACHE[which] = build_merge_nc() if which == "merge" else build_main_nc()
    return _NC_CACHE[which]


def run(inputs, trace=False, trace_cores=None):
    core_ids = list(range(NCORES))
    nc_a = _get_nc("merge")
    in_a = prep_merge_inputs(inputs["alphas"], inputs["W_down_all"],
                             inputs["W_up_all"], inputs["W_ln_all"],
                             inputs["b_ln_all"])
    res_a = run_bass_kernel_spmd(nc_a, in_a, core_ids=core_ids, trace=trace,
                                 trace_cores=trace_cores)
    wd8, wu8, pq = assemble_merge(res_a.results)

    nc_b = _get_nc("main")
    in_b = prep_main_inputs(inputs["x"], wd8, wu8, pq)
    res_b = run_bass_kernel_spmd(nc_b, in_b, core_ids=core_ids, trace=trace,
                                 trace_cores=trace_cores)
    out = assemble_output(res_b.results)
    return out, res_a, res_b


def kernel(**inputs):
    inputs = {k: np.asarray(v, dtype=np.float32) for k, v in inputs.items()}
    out, _, _ = run(inputs)
    return out


# revision 21
# speedup vs baseline: 1.0336x; 1.0336x over previous
"""MixAdapter: alpha-weighted adapter superposition + joint layernorm + bottleneck MLP.

Two SPMD launches on 8 NeuronCores (HW-calibrated engine assignment):

  Launch A ("merge"): fp16 adapter stacks sharded across cores (~5MB each).
    Scaled copies alpha_n*W_n run on ACT (20) and DVE (5); two parallel
    accumulation chains run on DVE (tensor_tensor, 2x mode) and gpsimd,
    combined at the end.  Host gathers the 0.8MB of merged params.

  Host folding (tiny): wdTw = W_ln*W_down scaled+quantized to fp8e4,
    wuT zero-padded/scaled/quantized, P/Q bias vectors.

  Launch B ("main"): batch elem k -> core k.
    - x^T fp16 in; ACT downcasts all 8 d-chunks to fp8 (x*32) with accum_out
      providing S1 (the sum).
    - S2: DVE squares x pairwise (tensor_tensor, 2x); PE ones-matmuls
      column-sum the squares into a PSUM accumulator; one small DVE reduce.
    - Down/up projections: fp8e4 DoubleRow matmuls with 1024-wide moving
      APs (512 output columns per instruction).
    - ReLU on ACT folds rstd/bias, requantizes h to fp8.
    - Residual y = psum/(WU*H) + x: dt 0-5 DVE stt from PSUM; dt 6-7 ACT
      scaled-evict + gpsimd add.  y written fp16, host upcasts.
"""

import numpy as np
import ml_dtypes

from concourse import bacc, mybir, tile
import concourse.bass as bass
from concourse.bass_utils import run_bass_kernel_spmd

B, S, D, BOT, N = 8, 2048, 1024, 400, 25
NCORES = 8
EPS = 1e-5
FP32 = mybir.dt.float32
F16 = mybir.dt.float16
F8 = mybir.dt.float8e4
U8 = mybir.dt.uint8
NP_F8 = ml_dtypes.float8_e4m3
F8_MAX = 240.0

DC = D // 128        # 8 d-chunks
OC = 4               # o-chunks (400 -> 3x128 + 16; padded to 512 for up-proj)
O_SZ = [128, 128, 128, 16]
NSBP = S // 512      # 4 psum-bank-wide moving groups

X_SCL = 32.0
W_SCL = 4096.0
WU_SCL = 1024.0
H_SCL = 64.0
PSD_INV = 1.0 / (W_SCL * X_SCL)
PSU_INV = 1.0 / (WU_SCL * H_SCL)

USE_F32R = False  # kept for test.py compatibility

WD_ROWS = BOT // NCORES
WU_ROWS = D // NCORES
MF = 400 + 400 + 2 * DC

DR = mybir.MatmulPerfMode.DoubleRow


# ---------------------------------------------------------------------------
# Launch A: alpha-weighted merge of the adapter stacks (sharded over cores)
# ---------------------------------------------------------------------------

N_ACT_COPY = 20   # adapters whose scaled copy runs on ACT (rest on DVE)
N_DVE_ACC = 16    # adapters 1..15 accumulate on DVE; 17.. on gpsimd


def build_merge_nc():
    nc = bacc.Bacc("TRN2", target_bir_lowering=False, debug=False,
                   enable_asserts=False, num_devices=NCORES)

    # adapters per stack DMA: tiny first group so the DVE chain starts early
    GRPS = [1, 4, 5, 5, 5, 5]
    stack = nc.dram_tensor("stack", [128, N * MF], F16, kind="ExternalInput")
    alphas = nc.dram_tensor("alphas", [1, N], FP32, kind="ExternalInput")
    out_m = nc.dram_tensor("out_m", [128, MF], F16, kind="ExternalOutput")

    with tile.TileContext(nc) as tc:
        with (
            tc.tile_pool(name="consts", bufs=1) as consts,
            tc.tile_pool(name="acc", bufs=1) as accp,
            tc.tile_pool(name="stk", bufs=3) as stk_pool,
            tc.tile_pool(name="psum", bufs=1, space="PSUM") as psum,
        ):
            a_sb = consts.tile([1, N], FP32)
            nc.sync.dma_start(a_sb[:], alphas[:])
            ones_row = consts.tile([1, 128], FP32)
            nc.vector.memset(ones_row[:], 1.0)
            pa = psum.tile([128, N], FP32)
            nc.tensor.matmul(pa[:], ones_row[:], a_sb[:], start=True, stop=True)
            a_bc = consts.tile([128, N], FP32)
            nc.scalar.copy(a_bc[:], pa[:])

            # DVE runs a fused copy+scale+add chain for adapters 0..12 while
            # ACT produces scaled copies of 13..24 that DVE then adds (2x tt).
            N_CHAIN = 13
            acc = accp.tile([128, MF], F16)
            tmps = []
            n0 = 0
            for g, grp in enumerate(GRPS):
                st = stk_pool.tile([128, grp * MF], F16, name=f"st{g}", tag="st")
                nc.sync.dma_start(st[:], stack[:, n0 * MF:(n0 + grp) * MF])
                for q in range(grp):
                    n = n0 + q
                    al = a_bc[:, n:n + 1]
                    sl = st[:, q * MF:(q + 1) * MF]
                    if n == 0:
                        nc.vector.tensor_scalar_mul(acc[:], sl, al)
                    elif n < N_CHAIN:
                        nc.vector.scalar_tensor_tensor(
                            acc[:], sl, al, acc[:],
                            mybir.AluOpType.mult, mybir.AluOpType.add)
                    else:
                        tm = accp.tile([128, MF], F16, name=f"tm{n}", tag=f"tm{n}")
                        nc.scalar.activation(tm[:], sl,
                                             mybir.ActivationFunctionType.Copy,
                                             scale=al)
                        tmps.append(tm)
                n0 += grp
            for tm in tmps:
                nc.vector.tensor_tensor(acc[:], acc[:], tm[:],
                                        mybir.AluOpType.add)

            nc.sync.dma_start(out_m[:], acc[:])

    nc.finalize()
    return nc


# ---------------------------------------------------------------------------
# Launch B: layernorm + down/up projections, one batch element per core
# ---------------------------------------------------------------------------

def build_main_nc():
    nc = bacc.Bacc("TRN2", target_bir_lowering=False, debug=False,
                   enable_asserts=False, num_devices=NCORES)

    xT16 = nc.dram_tensor("xT16", [128, DC, S], F16, kind="ExternalInput")
    wd8 = nc.dram_tensor("wd8", [128, DC, BOT], U8, kind="ExternalInput")
    wu8 = nc.dram_tensor("wu8", [128, OC, D], U8, kind="ExternalInput")
    pq = nc.dram_tensor("pq", [128, 2 * OC], FP32, kind="ExternalInput")
    yT = nc.dram_tensor("yT", [128, NSBP, DC, 512], F16, kind="ExternalOutput")

    inv1 = 1.0 / (X_SCL * float(S * D))   # S1 -> mu
    inv2 = 1.0 / float(S * D)             # S2 -> E[x^2]

    with tile.TileContext(nc) as tc:
        with (
            tc.tile_pool(name="xt", bufs=1) as xt_pool,
            tc.tile_pool(name="x8", bufs=1) as x8_pool,
            tc.tile_pool(name="ht", bufs=1) as ht_pool,
            tc.tile_pool(name="w", bufs=1) as w_pool,
            tc.tile_pool(name="small", bufs=1) as small,
            tc.tile_pool(name="sq", bufs=3) as sq_pool,
            tc.tile_pool(name="yo", bufs=4) as yo_pool,
            tc.tile_pool(name="pmd", bufs=2, space="PSUM") as pmd,
            tc.tile_pool(name="pmu", bufs=3, space="PSUM") as pmu,
            tc.tile_pool(name="pst", bufs=1, space="PSUM") as pstp,
        ):
            # ---- x stream: 4 chunk-pair DMAs; weights interleaved ----
            xt16 = []
            for j in range(DC // 2):
                t = xt_pool.tile([128, 2, S], F16, name=f"xt{j}", tag=f"xt{j}")
                nc.sync.dma_start(t[:], xT16[:, 2 * j:2 * j + 2, :])
                xt16.append(t)

            wd_sb = w_pool.tile([128, DC, BOT], F8, tag="wd")
            nc.sync.dma_start(wd_sb[:].bitcast(U8), wd8[:])
            wu_sb = w_pool.tile([128, OC, D], F8, tag="wu")
            nc.sync.dma_start(wu_sb[:].bitcast(U8), wu8[:])
            pq_sb = small.tile([128, 2 * OC], FP32)
            nc.sync.dma_start(pq_sb[:], pq[:])

            ht = [ht_pool.tile([128, 2, S], F8, name=f"ht{j}", tag=f"ht{j}")
                  for j in range(2)]
            nc.gpsimd.memset(ht[1][:, 1, :], 0.0)

            ones16 = small.tile([128, 1], F16)
            nc.vector.memset(ones16[:], 1.0)

            # stats PSUM bank: colsums of x^2 on partition 0, scalar matmul
            # outputs parked at other partitions/columns of the same bank
            pstc = pstp.tile([128, 512], FP32)
            stat_ps = pstc[0:1, 0:512]

            # ---- downcast (ACT, S1 via accum) + squares (DVE) + colsums (PE)
            x8 = []
            sums = small.tile([128, DC], FP32)
            for j in range(DC // 2):
                t8 = x8_pool.tile([128, 2, S], F8, name=f"x8{j}", tag=f"x8{j}")
                for i in range(2):
                    c = 2 * j + i
                    if c < 7:
                        nc.scalar.activation(t8[:, i, :], xt16[j][:, i, :],
                                             mybir.ActivationFunctionType.Copy,
                                             scale=X_SCL,
                                             accum_out=sums[:, c:c + 1])
                    else:
                        nc.vector.tensor_scalar(t8[:, i, :], xt16[j][:, i, :],
                                                X_SCL, 0.0,
                                                mybir.AluOpType.mult,
                                                mybir.AluOpType.add,
                                                accum_out=sums[:, c:c + 1])
                sq = sq_pool.tile([128, 2, S], F16, name=f"sq{j}", tag="sq")
                nc.vector.tensor_tensor(sq[:], xt16[j][:], xt16[j][:],
                                        mybir.AluOpType.mult)
                for m in range(8):
                    nc.tensor.matmul(stat_ps[:],
                                     ones16[:],
                                     sq[:, m // 4, 512 * (m % 4):512 * (m % 4 + 1)],
                                     start=(j == 0 and m == 0),
                                     stop=(j == 3 and m == 7))
                x8.append(t8)

            # ---- stats scalar chain ----
            s1 = small.tile([128, 1], FP32)
            nc.vector.tensor_reduce(s1[:], sums[:], mybir.AxisListType.X,
                                    mybir.AluOpType.add)
            inv1_col = small.tile([128, 1], FP32)
            nc.vector.memset(inv1_col[:], inv1)
            ones_row = small.tile([1, 128], FP32)
            nc.vector.memset(ones_row[:], 1.0)

            nc.tensor.matmul(pstc[32:33, 0:1], inv1_col[:], s1[:],
                             start=True, stop=True)

            sc = small.tile([1, 8], FP32)
            mu, s2r, e2, nvar, std, rstd, rs, mrn = (sc[:, i:i + 1] for i in range(8))
            mr = small.tile([1, 1], FP32)
            eps_sb = small.tile([1, 1], FP32)
            nc.vector.memset(eps_sb[:], EPS)
            nc.scalar.copy(mu, pstc[32:33, 0:1])
            nc.vector.tensor_reduce(s2r, stat_ps[:], mybir.AxisListType.X,
                                    mybir.AluOpType.add)
            nc.vector.tensor_scalar_mul(e2, s2r, inv2)
            # nvar = mu^2 - e2 ; std = sqrt(-nvar + eps) ; rstd = 1/std
            nc.vector.scalar_tensor_tensor(nvar, mu, mu, e2,
                                           mybir.AluOpType.mult,
                                           mybir.AluOpType.subtract)
            nc.scalar.activation(std, nvar, mybir.ActivationFunctionType.Sqrt,
                                 bias=eps_sb[:], scale=-1.0)
            nc.vector.reciprocal(rstd, std)
            nc.vector.tensor_scalar_mul(rs, rstd, H_SCL * PSD_INV)
            nc.vector.tensor_tensor(mr, mu, rstd, mybir.AluOpType.mult)
            nc.vector.tensor_scalar_mul(mrn, mr, -H_SCL)

            nc.tensor.matmul(pstc[:, 2:4], ones_row[:], sc[:, 6:8],
                             start=True, stop=True)
            bc = small.tile([128, 2], FP32)
            nc.scalar.copy(bc[:], pstc[:, 2:4])

            bias_sb = small.tile([128, OC], FP32)
            nc.vector.scalar_tensor_tensor(
                bias_sb[:], pq_sb[:, OC:2 * OC], bc[:, 1:2], pq_sb[:, 0:OC],
                mybir.AluOpType.mult, mybir.AluOpType.add)

            # ---- down-proj (fp8 DoubleRow, 1024-wide moving) + ReLU ----
            for ot in range(OC):
                osz = O_SZ[ot]
                for sbpp in range(NSBP // 2):
                    ph = pmd.tile([128, 1024], FP32, name=f"ph{ot}_{sbpp}", tag="mmd")
                    for half in range(2):
                        sbp = 2 * sbpp + half
                        for kk in range(4):
                            nc.tensor.matmul(
                                ph[:osz, 512 * half:512 * (half + 1)],
                                wd_sb[:, 2 * kk:2 * kk + 2, 128 * ot:128 * ot + osz],
                                x8[kk][:, :, 512 * sbp:512 * (sbp + 1)],
                                start=(kk == 0), stop=(kk == 3), perf_mode=DR)
                    nc.scalar.activation(
                        ht[ot // 2][:osz, ot % 2, 1024 * sbpp:1024 * (sbpp + 1)],
                        ph[:osz, :],
                        mybir.ActivationFunctionType.Relu,
                        bias=bias_sb[:osz, ot:ot + 1], scale=bc[:osz, 0:1])

            # ---- up-proj (fp8 DoubleRow) + residual + store ----
            for sbp in range(NSBP):
                yo = yo_pool.tile([128, DC, 512], F16, name=f"yo{sbp}", tag="yo")
                for dt in range(DC):
                    pu = pmu.tile([128, 512], FP32, name=f"pu{dt}_{sbp}", tag="mmu")
                    for kk in range(2):
                        nc.tensor.matmul(
                            pu[:],
                            wu_sb[:, 2 * kk:2 * kk + 2, 128 * dt:128 * (dt + 1)],
                            ht[kk][:, :, 512 * sbp:512 * (sbp + 1)],
                            start=(kk == 0), stop=(kk == 1), perf_mode=DR)
                    xs = xt16[dt // 2][:, dt % 2, 512 * sbp:512 * (sbp + 1)]
                    if dt < 6:
                        nc.vector.scalar_tensor_tensor(
                            yo[:, dt, :], pu[:], PSU_INV, xs,
                            mybir.AluOpType.mult, mybir.AluOpType.add)
                    else:
                        nc.scalar.activation(yo[:, dt, :], pu[:],
                                             mybir.ActivationFunctionType.Copy,
                                             scale=PSU_INV)
                        nc.gpsimd.tensor_tensor(yo[:, dt, :], yo[:, dt, :], xs,
                                                mybir.AluOpType.add)
                nc.sync.dma_start(yT[:, sbp, :, :], yo[:])

    nc.finalize()
    return nc


# ---------------------------------------------------------------------------
# Host-side orchestration
# ---------------------------------------------------------------------------

def prep_merge_inputs(alphas, W_down_all, W_up_all, W_ln_all, b_ln_all):
    a_in = np.ascontiguousarray(alphas.reshape(1, N)).astype(np.float32)
    wln = W_ln_all.reshape(N, DC, 128).transpose(0, 2, 1)
    bln = b_ln_all.reshape(N, DC, 128).transpose(0, 2, 1)
    ln_blk = np.concatenate([wln, bln], axis=2)             # [N,128,16]
    in_maps = []
    for k in range(NCORES):
        wd_k = W_down_all[:, WD_ROWS * k:WD_ROWS * (k + 1), :].reshape(N, 128, 400)
        wu_k = W_up_all[:, WU_ROWS * k:WU_ROWS * (k + 1), :]
        stack = np.concatenate([wd_k, wu_k, ln_blk], axis=2).astype(np.float16)
        # all adapters side-by-side in the free dim: [128, N*MF]
        stack = stack.transpose(1, 0, 2).reshape(128, N * MF)
        in_maps.append({"stack": np.ascontiguousarray(stack), "alphas": a_in})
    return in_maps


def _to_f8(a):
    return np.clip(a, -F8_MAX, F8_MAX).astype(NP_F8)


def assemble_merge(results):
    W_down = np.concatenate(
        [results[k]["out_m"][:, 0:400].astype(np.float32).reshape(WD_ROWS, D)
         for k in range(NCORES)], axis=0)                   # [BOT, D]
    W_up = np.concatenate(
        [results[k]["out_m"][:, 400:800].astype(np.float32)
         for k in range(NCORES)], axis=0)                   # [D, BOT]
    ln = results[0]["out_m"][:, 800:].astype(np.float32)
    W_ln = ln[:, 0:DC].T.reshape(D)
    b_ln = ln[:, DC:2 * DC].T.reshape(D)

    wdT = W_down.T * (W_ln * W_SCL)[:, None]
    wd8 = _to_f8(wdT.reshape(DC, 128, BOT).transpose(1, 0, 2))

    wuT_pad = np.zeros((4 * 128, D), dtype=np.float32)
    wuT_pad[:BOT] = W_up.T * WU_SCL
    wu8 = _to_f8(wuT_pad.reshape(OC, 128, D).transpose(1, 0, 2))

    P = W_down @ b_ln
    Q = W_down @ W_ln
    pq = np.zeros((128, 2 * OC), dtype=np.float32)
    Pp = np.zeros(512, dtype=np.float32); Pp[:BOT] = H_SCL * P
    Qp = np.zeros(512, dtype=np.float32); Qp[:BOT] = Q
    pq[:, 0:OC] = Pp.reshape(OC, 128).T
    pq[:, OC:2 * OC] = Qp.reshape(OC, 128).T
    return (np.ascontiguousarray(wd8).view(np.uint8),
            np.ascontiguousarray(wu8).view(np.uint8),
            np.ascontiguousarray(pq))


def prep_main_inputs(x, wd8, wu8, pq):
    in_maps = []
    for k in range(NCORES):
        xt = x[k].T.reshape(DC, 128, S).transpose(1, 0, 2).astype(np.float16)
        in_maps.append({"xT16": np.ascontiguousarray(xt),
                        "wd8": wd8, "wu8": wu8, "pq": pq})
    return in_maps


def assemble_output(results):
    out = np.empty((B, S, D), dtype=np.float32)
    for k in range(NCORES):
        y = results[k]["yT"].astype(np.float32)   # [128, NSBP, DC, 512]
        out[k] = y.transpose(1, 3, 2, 0).reshape(S, D)
    return out


_NC_CACHE = {}


def _get_nc(which):
    if which not in _NC_CACHE:
        _NC_CACHE[which] = build_merge_nc() if which == "merge" else build_main_nc()
    return _NC_CACHE[which]


def run(inputs, trace=False, trace_cores=None):
    core_ids = list(range(NCORES))
    nc_a = _get_nc("merge")
    in_a = prep_merge_inputs(inputs["alphas"], inputs["W_down_all"],
                             inputs["W_up_all"], inputs["W_ln_all"],
                             inputs["b_ln_all"])
    res_a = run_bass_kernel_spmd(nc_a, in_a, core_ids=core_ids, trace=trace,
                                 trace_cores=trace_cores)
    wd8, wu8, pq = assemble_merge(res_a.results)

    nc_b = _get_nc("main")
    in_b = prep_main_inputs(inputs["x"], wd8, wu8, pq)
    res_b = run_bass_kernel_spmd(nc_b, in_b, core_ids=core_ids, trace=trace,
                                 trace_cores=trace_cores)
    out = assemble_output(res_b.results)
    return out, res_a, res_b


def kernel(**inputs):
    inputs = {k: np.asarray(v, dtype=np.float32) for k, v in inputs.items()}
    out, _, _ = run(inputs)
    return out


# revision 22
# speedup vs baseline: 1.1022x; 1.0664x over previous
"""MixAdapter: alpha-weighted adapter superposition + joint layernorm + bottleneck MLP.

Two SPMD launches on 8 NeuronCores (HW-calibrated engine assignment):

  Launch A ("merge"): fp16 adapter stacks sharded across cores (~5MB each).
    Scaled copies alpha_n*W_n run on ACT (20) and DVE (5); two parallel
    accumulation chains run on DVE (tensor_tensor, 2x mode) and gpsimd,
    combined at the end.  Host gathers the 0.8MB of merged params.

  Host folding (tiny): wdTw = W_ln*W_down scaled+quantized to fp8e4,
    wuT zero-padded/scaled/quantized, P/Q bias vectors.

  Launch B ("main"): batch elem k -> core k.
    - x^T fp16 in; ACT downcasts all 8 d-chunks to fp8 (x*32) with accum_out
      providing S1 (the sum).
    - S2: DVE squares x pairwise (tensor_tensor, 2x); PE ones-matmuls
      column-sum the squares into a PSUM accumulator; one small DVE reduce.
    - Down/up projections: fp8e4 DoubleRow matmuls with 1024-wide moving
      APs (512 output columns per instruction).
    - ReLU on ACT folds rstd/bias, requantizes h to fp8.
    - Residual y = psum/(WU*H) + x: dt 0-5 DVE stt from PSUM; dt 6-7 ACT
      scaled-evict + gpsimd add.  y written fp16, host upcasts.
"""

import numpy as np
import ml_dtypes

from concourse import bacc, mybir, tile
import concourse.bass as bass
from concourse.bass_utils import run_bass_kernel_spmd

B, S, D, BOT, N = 8, 2048, 1024, 400, 25
NCORES = 8
EPS = 1e-5
FP32 = mybir.dt.float32
F16 = mybir.dt.float16
F8 = mybir.dt.float8e4
U8 = mybir.dt.uint8
NP_F8 = ml_dtypes.float8_e4m3
F8_MAX = 240.0

DC = D // 128        # 8 d-chunks
OC = 4               # o-chunks (400 -> 3x128 + 16; padded to 512 for up-proj)
O_SZ = [128, 128, 128, 16]
NSBP = S // 512      # 4 psum-bank-wide moving groups

X_SCL = 32.0
W_SCL = 4096.0
WU_SCL = 1024.0
H_SCL = 64.0
PSD_INV = 1.0 / (W_SCL * X_SCL)
PSU_INV = 1.0 / (WU_SCL * H_SCL)

USE_F32R = False  # kept for test.py compatibility

WD_ROWS = BOT // NCORES
WU_ROWS = D // NCORES
MF = 400 + 400 + 2 * DC

DR = mybir.MatmulPerfMode.DoubleRow


# ---------------------------------------------------------------------------
# Launch A: alpha-weighted merge of the adapter stacks (sharded over cores)
# ---------------------------------------------------------------------------

N_ACT_COPY = 20   # adapters whose scaled copy runs on ACT (rest on DVE)
N_DVE_ACC = 16    # adapters 1..15 accumulate on DVE; 17.. on gpsimd


def build_merge_nc():
    nc = bacc.Bacc("TRN2", target_bir_lowering=False, debug=False,
                   enable_asserts=False, num_devices=NCORES)

    # adapters per stack DMA: tiny first group so the DVE chain starts early
    GRPS = [1, 4, 5, 5, 5, 5]
    stack = nc.dram_tensor("stack", [128, N * MF], U8, kind="ExternalInput")
    alphas = nc.dram_tensor("alphas", [1, N], FP32, kind="ExternalInput")
    out_m = nc.dram_tensor("out_m", [128, MF], F16, kind="ExternalOutput")

    with tile.TileContext(nc) as tc:
        with (
            tc.tile_pool(name="consts", bufs=1) as consts,
            tc.tile_pool(name="acc", bufs=1) as accp,
            tc.tile_pool(name="stk", bufs=3) as stk_pool,
            tc.tile_pool(name="psum", bufs=1, space="PSUM") as psum,
        ):
            a_sb = consts.tile([1, N], FP32)
            nc.sync.dma_start(a_sb[:], alphas[:])
            ones_row = consts.tile([1, 128], FP32)
            nc.vector.memset(ones_row[:], 1.0)
            pa = psum.tile([128, N], FP32)
            nc.tensor.matmul(pa[:], ones_row[:], a_sb[:], start=True, stop=True)
            a_bc = consts.tile([128, N], FP32)
            nc.scalar.copy(a_bc[:], pa[:])

            # DVE runs a fused copy+scale+add chain for adapters 0..12 while
            # ACT produces scaled copies of 13..24 that DVE then adds (2x tt).
            N_CHAIN = 13
            acc = accp.tile([128, MF], F16)
            tmps = []
            n0 = 0
            for g, grp in enumerate(GRPS):
                st = stk_pool.tile([128, grp * MF], F8, name=f"st{g}", tag="st")
                nc.sync.dma_start(st[:].bitcast(U8),
                                  stack[:, n0 * MF:(n0 + grp) * MF])
                for q in range(grp):
                    n = n0 + q
                    al = a_bc[:, n:n + 1]
                    sl = st[:, q * MF:(q + 1) * MF]
                    if n == 0:
                        nc.vector.tensor_scalar_mul(acc[:], sl, al)
                    elif n < N_CHAIN:
                        nc.vector.scalar_tensor_tensor(
                            acc[:], sl, al, acc[:],
                            mybir.AluOpType.mult, mybir.AluOpType.add)
                    else:
                        tm = accp.tile([128, MF], F16, name=f"tm{n}", tag=f"tm{n}")
                        nc.scalar.activation(tm[:], sl,
                                             mybir.ActivationFunctionType.Copy,
                                             scale=al)
                        tmps.append(tm)
                n0 += grp
            for tm in tmps:
                nc.vector.tensor_tensor(acc[:], acc[:], tm[:],
                                        mybir.AluOpType.add)

            nc.sync.dma_start(out_m[:], acc[:])

    nc.finalize()
    return nc


# ---------------------------------------------------------------------------
# Launch B: layernorm + down/up projections, one batch element per core
# ---------------------------------------------------------------------------

def build_main_nc():
    nc = bacc.Bacc("TRN2", target_bir_lowering=False, debug=False,
                   enable_asserts=False, num_devices=NCORES)

    xT16 = nc.dram_tensor("xT16", [128, DC, S], F16, kind="ExternalInput")
    wd8 = nc.dram_tensor("wd8", [128, DC, BOT], U8, kind="ExternalInput")
    wu8 = nc.dram_tensor("wu8", [128, OC, D], U8, kind="ExternalInput")
    pq = nc.dram_tensor("pq", [128, 2 * OC], FP32, kind="ExternalInput")
    yT = nc.dram_tensor("yT", [128, NSBP, DC, 512], F16, kind="ExternalOutput")

    inv1 = 1.0 / (X_SCL * float(S * D))   # S1 -> mu
    inv2 = 1.0 / float(S * D)             # S2 -> E[x^2]

    with tile.TileContext(nc) as tc:
        with (
            tc.tile_pool(name="xt", bufs=1) as xt_pool,
            tc.tile_pool(name="x8", bufs=1) as x8_pool,
            tc.tile_pool(name="ht", bufs=1) as ht_pool,
            tc.tile_pool(name="w", bufs=1) as w_pool,
            tc.tile_pool(name="small", bufs=1) as small,
            tc.tile_pool(name="sq", bufs=3) as sq_pool,
            tc.tile_pool(name="yo", bufs=4) as yo_pool,
            tc.tile_pool(name="pmd", bufs=2, space="PSUM") as pmd,
            tc.tile_pool(name="pmu", bufs=3, space="PSUM") as pmu,
            tc.tile_pool(name="pst", bufs=1, space="PSUM") as pstp,
        ):
            # ---- x stream: 4 chunk-pair DMAs; weights interleaved ----
            xt16 = []
            for j in range(DC // 2):
                t = xt_pool.tile([128, 2, S], F16, name=f"xt{j}", tag=f"xt{j}")
                nc.sync.dma_start(t[:], xT16[:, 2 * j:2 * j + 2, :])
                xt16.append(t)

            wd_sb = w_pool.tile([128, DC, BOT], F8, tag="wd")
            nc.sync.dma_start(wd_sb[:].bitcast(U8), wd8[:])
            wu_sb = w_pool.tile([128, OC, D], F8, tag="wu")
            nc.sync.dma_start(wu_sb[:].bitcast(U8), wu8[:])
            pq_sb = small.tile([128, 2 * OC], FP32)
            nc.sync.dma_start(pq_sb[:], pq[:])

            ht = [ht_pool.tile([128, 2, S], F8, name=f"ht{j}", tag=f"ht{j}")
                  for j in range(2)]
            nc.gpsimd.memset(ht[1][:, 1, :], 0.0)

            ones16 = small.tile([128, 1], F16)
            nc.vector.memset(ones16[:], 1.0)

            # stats PSUM bank: colsums of x^2 on partition 0, scalar matmul
            # outputs parked at other partitions/columns of the same bank
            pstc = pstp.tile([128, 512], FP32)
            stat_ps = pstc[0:1, 0:512]

            # ---- downcast (ACT, S1 via accum) + squares (DVE) + colsums (PE)
            x8 = []
            sums = small.tile([128, 5], FP32)
            for j in range(DC // 2):
                t8 = x8_pool.tile([128, 2, S], F8, name=f"x8{j}", tag=f"x8{j}")
                if j < 3:
                    # whole-pair downcast in one ACT instr (S1 per pair)
                    nc.scalar.activation(t8[:], xt16[j][:],
                                         mybir.ActivationFunctionType.Copy,
                                         scale=X_SCL,
                                         accum_out=sums[:, j:j + 1])
                else:
                    nc.scalar.activation(t8[:, 0, :], xt16[j][:, 0, :],
                                         mybir.ActivationFunctionType.Copy,
                                         scale=X_SCL,
                                         accum_out=sums[:, 3:4])
                    nc.vector.tensor_scalar(t8[:, 1, :], xt16[j][:, 1, :],
                                            X_SCL, 0.0,
                                            mybir.AluOpType.mult,
                                            mybir.AluOpType.add,
                                            accum_out=sums[:, 4:5])
                sq = sq_pool.tile([128, 2, S], F16, name=f"sq{j}", tag="sq")
                nc.vector.tensor_tensor(sq[:], xt16[j][:], xt16[j][:],
                                        mybir.AluOpType.mult)
                for m in range(8):
                    nc.tensor.matmul(stat_ps[:],
                                     ones16[:],
                                     sq[:, m // 4, 512 * (m % 4):512 * (m % 4 + 1)],
                                     start=(j == 0 and m == 0),
                                     stop=(j == 3 and m == 7))
                x8.append(t8)

            # ---- stats scalar chain ----
            s1 = small.tile([128, 1], FP32)
            nc.vector.tensor_reduce(s1[:], sums[:], mybir.AxisListType.X,
                                    mybir.AluOpType.add)
            inv1_col = small.tile([128, 1], FP32)
            nc.vector.memset(inv1_col[:], inv1)
            ones_row = small.tile([1, 128], FP32)
            nc.vector.memset(ones_row[:], 1.0)

            nc.tensor.matmul(pstc[32:33, 0:1], inv1_col[:], s1[:],
                             start=True, stop=True)

            sc = small.tile([1, 8], FP32)
            mu, s2r, e2, nvar, std, rstd, rs, mrn = (sc[:, i:i + 1] for i in range(8))
            mr = small.tile([1, 1], FP32)
            eps_sb = small.tile([1, 1], FP32)
            nc.vector.memset(eps_sb[:], EPS)
            nc.scalar.copy(mu, pstc[32:33, 0:1])
            nc.vector.tensor_reduce(s2r, stat_ps[:], mybir.AxisListType.X,
                                    mybir.AluOpType.add)
            nc.vector.tensor_scalar_mul(e2, s2r, inv2)
            # nvar = mu^2 - e2 ; std = sqrt(-nvar + eps) ; rstd = 1/std
            nc.vector.scalar_tensor_tensor(nvar, mu, mu, e2,
                                           mybir.AluOpType.mult,
                                           mybir.AluOpType.subtract)
            nc.scalar.activation(std, nvar, mybir.ActivationFunctionType.Sqrt,
                                 bias=eps_sb[:], scale=-1.0)
            nc.vector.reciprocal(rstd, std)
            nc.vector.tensor_scalar_mul(rs, rstd, H_SCL * PSD_INV)
            nc.vector.tensor_tensor(mr, mu, rstd, mybir.AluOpType.mult)
            nc.vector.tensor_scalar_mul(mrn, mr, -H_SCL)

            nc.tensor.matmul(pstc[:, 2:4], ones_row[:], sc[:, 6:8],
                             start=True, stop=True)
            bc = small.tile([128, 2], FP32)
            nc.scalar.copy(bc[:], pstc[:, 2:4])

            bias_sb = small.tile([128, OC], FP32)
            nc.vector.scalar_tensor_tensor(
                bias_sb[:], pq_sb[:, OC:2 * OC], bc[:, 1:2], pq_sb[:, 0:OC],
                mybir.AluOpType.mult, mybir.AluOpType.add)

            # ---- down-proj (fp8 DoubleRow, 1024-wide moving) + ReLU ----
            for ot in range(OC):
                osz = O_SZ[ot]
                for sbpp in range(NSBP // 2):
                    ph = pmd.tile([128, 1024], FP32, name=f"ph{ot}_{sbpp}", tag="mmd")
                    for half in range(2):
                        sbp = 2 * sbpp + half
                        for kk in range(4):
                            nc.tensor.matmul(
                                ph[:osz, 512 * half:512 * (half + 1)],
                                wd_sb[:, 2 * kk:2 * kk + 2, 128 * ot:128 * ot + osz],
                                x8[kk][:, :, 512 * sbp:512 * (sbp + 1)],
                                start=(kk == 0), stop=(kk == 3), perf_mode=DR)
                    nc.scalar.activation(
                        ht[ot // 2][:osz, ot % 2, 1024 * sbpp:1024 * (sbpp + 1)],
                        ph[:osz, :],
                        mybir.ActivationFunctionType.Relu,
                        bias=bias_sb[:osz, ot:ot + 1], scale=bc[:osz, 0:1])

            # ---- up-proj (fp8 DoubleRow) + residual + store ----
            for sbp in range(NSBP):
                yo = yo_pool.tile([128, DC, 512], F16, name=f"yo{sbp}", tag="yo")
                for dt in range(DC):
                    pu = pmu.tile([128, 512], FP32, name=f"pu{dt}_{sbp}", tag="mmu")
                    for kk in range(2):
                        nc.tensor.matmul(
                            pu[:],
                            wu_sb[:, 2 * kk:2 * kk + 2, 128 * dt:128 * (dt + 1)],
                            ht[kk][:, :, 512 * sbp:512 * (sbp + 1)],
                            start=(kk == 0), stop=(kk == 1), perf_mode=DR)
                    xs = xt16[dt // 2][:, dt % 2, 512 * sbp:512 * (sbp + 1)]
                    if dt < 6:
                        nc.vector.scalar_tensor_tensor(
                            yo[:, dt, :], pu[:], PSU_INV, xs,
                            mybir.AluOpType.mult, mybir.AluOpType.add)
                    else:
                        nc.scalar.activation(yo[:, dt, :], pu[:],
                                             mybir.ActivationFunctionType.Copy,
                                             scale=PSU_INV)
                        nc.gpsimd.tensor_tensor(yo[:, dt, :], yo[:, dt, :], xs,
                                                mybir.AluOpType.add)
                nc.sync.dma_start(yT[:, sbp, 0:4, :], yo[:, 0:4, :])
                nc.sync.dma_start(yT[:, sbp, 4:8, :], yo[:, 4:8, :])

    nc.finalize()
    return nc


# ---------------------------------------------------------------------------
# Host-side orchestration
# ---------------------------------------------------------------------------

def prep_merge_inputs(alphas, W_down_all, W_up_all, W_ln_all, b_ln_all):
    a_in = np.ascontiguousarray(alphas.reshape(1, N)).astype(np.float32)
    wln = W_ln_all.reshape(N, DC, 128).transpose(0, 2, 1)
    bln = b_ln_all.reshape(N, DC, 128).transpose(0, 2, 1)
    ln_blk = np.concatenate([wln, bln], axis=2)             # [N,128,16]
    in_maps = []
    for k in range(NCORES):
        wd_k = W_down_all[:, WD_ROWS * k:WD_ROWS * (k + 1), :].reshape(N, 128, 400)
        wu_k = W_up_all[:, WU_ROWS * k:WU_ROWS * (k + 1), :]
        stack = np.concatenate([wd_k, wu_k, ln_blk], axis=2)
        # all adapters side-by-side in the free dim, fp8e4 at x64 scale
        stack = stack.transpose(1, 0, 2).reshape(128, N * MF)
        stack = _to_f8(stack * 64.0)
        in_maps.append({"stack": np.ascontiguousarray(stack).view(np.uint8),
                        "alphas": a_in})
    return in_maps


def _to_f8(a):
    return np.clip(a, -F8_MAX, F8_MAX).astype(NP_F8)


def assemble_merge(results):
    W_down = np.concatenate(
        [results[k]["out_m"][:, 0:400].astype(np.float32).reshape(WD_ROWS, D)
         for k in range(NCORES)], axis=0) / 64.0            # [BOT, D]
    W_up = np.concatenate(
        [results[k]["out_m"][:, 400:800].astype(np.float32)
         for k in range(NCORES)], axis=0) / 64.0            # [D, BOT]
    ln = results[0]["out_m"][:, 800:].astype(np.float32) / 64.0
    W_ln = ln[:, 0:DC].T.reshape(D)
    b_ln = ln[:, DC:2 * DC].T.reshape(D)

    wdT = W_down.T * (W_ln * W_SCL)[:, None]
    wd8 = _to_f8(wdT.reshape(DC, 128, BOT).transpose(1, 0, 2))

    wuT_pad = np.zeros((4 * 128, D), dtype=np.float32)
    wuT_pad[:BOT] = W_up.T * WU_SCL
    wu8 = _to_f8(wuT_pad.reshape(OC, 128, D).transpose(1, 0, 2))

    P = W_down @ b_ln
    Q = W_down @ W_ln
    pq = np.zeros((128, 2 * OC), dtype=np.float32)
    Pp = np.zeros(512, dtype=np.float32); Pp[:BOT] = H_SCL * P
    Qp = np.zeros(512, dtype=np.float32); Qp[:BOT] = Q
    pq[:, 0:OC] = Pp.reshape(OC, 128).T
    pq[:, OC:2 * OC] = Qp.reshape(OC, 128).T
    return (np.ascontiguousarray(wd8).view(np.uint8),
            np.ascontiguousarray(wu8).view(np.uint8),
            np.ascontiguousarray(pq))


def prep_main_inputs(x, wd8, wu8, pq):
    in_maps = []
    for k in range(NCORES):
        xt = x[k].T.reshape(DC, 128, S).transpose(1, 0, 2).astype(np.float16)
        in_maps.append({"xT16": np.ascontiguousarray(xt),
                        "wd8": wd8, "wu8": wu8, "pq": pq})
    return in_maps


def assemble_output(results):
    out = np.empty((B, S, D), dtype=np.float32)
    for k in range(NCORES):
        y = results[k]["yT"].astype(np.float32)   # [128, NSBP, DC, 512]
        out[k] = y.transpose(1, 3, 2, 0).reshape(S, D)
    return out


_NC_CACHE = {}


def _get_nc(which):
    if which not in _NC_CACHE:
        _NC_CACHE[which] = build_merge_nc() if which == "merge" else build_main_nc()
    return _NC_CACHE[which]


def run(inputs, trace=False, trace_cores=None):
    core_ids = list(range(NCORES))
    nc_a = _get_nc("merge")
    in_a = prep_merge_inputs(inputs["alphas"], inputs["W_down_all"],
                             inputs["W_up_all"], inputs["W_ln_all"],
                             inputs["b_ln_all"])
    res_a = run_bass_kernel_spmd(nc_a, in_a, core_ids=core_ids, trace=trace,
                                 trace_cores=trace_cores)
    wd8, wu8, pq = assemble_merge(res_a.results)

    nc_b = _get_nc("main")
    in_b = prep_main_inputs(inputs["x"], wd8, wu8, pq)
    res_b = run_bass_kernel_spmd(nc_b, in_b, core_ids=core_ids, trace=trace,
                                 trace_cores=trace_cores)
    out = assemble_output(res_b.results)
    return out, res_a, res_b


def kernel(**inputs):
    inputs = {k: np.asarray(v, dtype=np.float32) for k, v in inputs.items()}
    out, _, _ = run(inputs)
    return out


# revision 23
# speedup vs baseline: 1.1361x; 1.0308x over previous
"""MixAdapter: alpha-weighted adapter superposition + joint layernorm + bottleneck MLP.

Two SPMD launches on 8 NeuronCores (HW-calibrated engine assignment):

  Launch A ("merge"): fp16 adapter stacks sharded across cores (~5MB each).
    Scaled copies alpha_n*W_n run on ACT (20) and DVE (5); two parallel
    accumulation chains run on DVE (tensor_tensor, 2x mode) and gpsimd,
    combined at the end.  Host gathers the 0.8MB of merged params.

  Host folding (tiny): wdTw = W_ln*W_down scaled+quantized to fp8e4,
    wuT zero-padded/scaled/quantized, P/Q bias vectors.

  Launch B ("main"): batch elem k -> core k.
    - x^T fp16 in; ACT downcasts all 8 d-chunks to fp8 (x*32) with accum_out
      providing S1 (the sum).
    - S2: DVE squares x pairwise (tensor_tensor, 2x); PE ones-matmuls
      column-sum the squares into a PSUM accumulator; one small DVE reduce.
    - Down/up projections: fp8e4 DoubleRow matmuls with 1024-wide moving
      APs (512 output columns per instruction).
    - ReLU on ACT folds rstd/bias, requantizes h to fp8.
    - Residual y = psum/(WU*H) + x: dt 0-5 DVE stt from PSUM; dt 6-7 ACT
      scaled-evict + gpsimd add.  y written fp16, host upcasts.
"""

import numpy as np
import ml_dtypes

from concourse import bacc, mybir, tile
import concourse.bass as bass
from concourse.bass_utils import run_bass_kernel_spmd

B, S, D, BOT, N = 8, 2048, 1024, 400, 25
NCORES = 8
EPS = 1e-5
FP32 = mybir.dt.float32
F16 = mybir.dt.float16
F8 = mybir.dt.float8e4
U8 = mybir.dt.uint8
NP_F8 = ml_dtypes.float8_e4m3
F8_MAX = 240.0

DC = D // 128        # 8 d-chunks
OC = 4               # o-chunks (400 -> 3x128 + 16; padded to 512 for up-proj)
O_SZ = [128, 128, 128, 16]
NSBP = S // 512      # 4 psum-bank-wide moving groups

X_SCL = 32.0
W_SCL = 4096.0
WU_SCL = 1024.0
H_SCL = 64.0
PSD_INV = 1.0 / (W_SCL * X_SCL)
PSU_INV = 1.0 / (WU_SCL * H_SCL)

USE_F32R = False  # kept for test.py compatibility

WD_ROWS = BOT // NCORES
WU_ROWS = D // NCORES
MF = 400 + 400 + 2 * DC

DR = mybir.MatmulPerfMode.DoubleRow


# ---------------------------------------------------------------------------
# Launch A: alpha-weighted merge of the adapter stacks (sharded over cores)
# ---------------------------------------------------------------------------

N_ACT_COPY = 20   # adapters whose scaled copy runs on ACT (rest on DVE)
N_DVE_ACC = 16    # adapters 1..15 accumulate on DVE; 17.. on gpsimd


def build_merge_nc():
    nc = bacc.Bacc("TRN2", target_bir_lowering=False, debug=False,
                   enable_asserts=False, num_devices=NCORES)

    # adapters per stack DMA: tiny first group so the DVE chain starts early
    GRPS = [1, 4, 5, 5, 5, 5]
    stack = nc.dram_tensor("stack", [128, N * MF], U8, kind="ExternalInput")
    alphas = nc.dram_tensor("alphas", [1, N], FP32, kind="ExternalInput")
    out_m = nc.dram_tensor("out_m", [128, MF], F16, kind="ExternalOutput")

    with tile.TileContext(nc) as tc:
        with (
            tc.tile_pool(name="consts", bufs=1) as consts,
            tc.tile_pool(name="acc", bufs=1) as accp,
            tc.tile_pool(name="stk", bufs=3) as stk_pool,
            tc.tile_pool(name="psum", bufs=1, space="PSUM") as psum,
        ):
            a_sb = consts.tile([1, N], FP32)
            nc.sync.dma_start(a_sb[:], alphas[:])
            ones_row = consts.tile([1, 128], FP32)
            nc.vector.memset(ones_row[:], 1.0)
            pa = psum.tile([128, N], FP32)
            nc.tensor.matmul(pa[:], ones_row[:], a_sb[:], start=True, stop=True)
            a_bc = consts.tile([128, N], FP32)
            nc.scalar.copy(a_bc[:], pa[:])

            # DVE runs a fused copy+scale+add chain for adapters 0..12 while
            # ACT produces scaled copies of 13..24 that DVE then adds (2x tt).
            N_CHAIN = 9
            acc = accp.tile([128, MF], F16)
            tmps = []
            n0 = 0
            for g, grp in enumerate(GRPS):
                st = stk_pool.tile([128, grp * MF], F8, name=f"st{g}", tag="st")
                nc.sync.dma_start(st[:].bitcast(U8),
                                  stack[:, n0 * MF:(n0 + grp) * MF])
                for q in range(grp):
                    n = n0 + q
                    al = a_bc[:, n:n + 1]
                    sl = st[:, q * MF:(q + 1) * MF]
                    if n == 0:
                        nc.vector.tensor_scalar_mul(acc[:], sl, al)
                    elif n < N_CHAIN:
                        nc.vector.scalar_tensor_tensor(
                            acc[:], sl, al, acc[:],
                            mybir.AluOpType.mult, mybir.AluOpType.add)
                    else:
                        tm = accp.tile([128, MF], F16, name=f"tm{n}", tag=f"tm{n}")
                        nc.scalar.activation(tm[:], sl,
                                             mybir.ActivationFunctionType.Copy,
                                             scale=al)
                        tmps.append(tm)
                n0 += grp
            for tm in tmps:
                nc.vector.tensor_tensor(acc[:], acc[:], tm[:],
                                        mybir.AluOpType.add)

            nc.sync.dma_start(out_m[:], acc[:])

    nc.finalize()
    return nc


# ---------------------------------------------------------------------------
# Launch B: layernorm + down/up projections, one batch element per core
# ---------------------------------------------------------------------------

def build_main_nc():
    nc = bacc.Bacc("TRN2", target_bir_lowering=False, debug=False,
                   enable_asserts=False, num_devices=NCORES)

    xT16 = nc.dram_tensor("xT16", [128, DC, S], F16, kind="ExternalInput")
    wd8 = nc.dram_tensor("wd8", [128, DC, BOT], U8, kind="ExternalInput")
    wu8 = nc.dram_tensor("wu8", [128, OC, D], U8, kind="ExternalInput")
    pq = nc.dram_tensor("pq", [128, 2 * OC], FP32, kind="ExternalInput")
    yT = nc.dram_tensor("yT", [128, NSBP, DC, 512], F16, kind="ExternalOutput")

    inv1 = 1.0 / (X_SCL * float(S * D))   # S1 -> mu
    inv2 = 1.0 / float(S * D)             # S2 -> E[x^2]

    with tile.TileContext(nc) as tc:
        with (
            tc.tile_pool(name="xt", bufs=1) as xt_pool,
            tc.tile_pool(name="x8", bufs=1) as x8_pool,
            tc.tile_pool(name="ht", bufs=1) as ht_pool,
            tc.tile_pool(name="w", bufs=1) as w_pool,
            tc.tile_pool(name="small", bufs=1) as small,
            tc.tile_pool(name="sq", bufs=3) as sq_pool,
            tc.tile_pool(name="yo", bufs=4) as yo_pool,
            tc.tile_pool(name="pmd", bufs=2, space="PSUM") as pmd,
            tc.tile_pool(name="pmu", bufs=3, space="PSUM") as pmu,
            tc.tile_pool(name="pst", bufs=1, space="PSUM") as pstp,
        ):
            # ---- x stream: 4 chunk-pair DMAs; weights interleaved ----
            xt16 = []
            for j in range(DC // 2):
                t = xt_pool.tile([128, 2, S], F16, name=f"xt{j}", tag=f"xt{j}")
                nc.sync.dma_start(t[:], xT16[:, 2 * j:2 * j + 2, :])
                xt16.append(t)

            wd_sb = w_pool.tile([128, DC, BOT], F8, tag="wd")
            nc.sync.dma_start(wd_sb[:].bitcast(U8), wd8[:])
            wu_sb = w_pool.tile([128, OC, D], F8, tag="wu")
            nc.sync.dma_start(wu_sb[:].bitcast(U8), wu8[:])
            pq_sb = small.tile([128, 2 * OC], FP32)
            nc.sync.dma_start(pq_sb[:], pq[:])

            ht = [ht_pool.tile([128, 2, S], F8, name=f"ht{j}", tag=f"ht{j}")
                  for j in range(2)]
            nc.gpsimd.memset(ht[1][:, 1, :], 0.0)

            ones16 = small.tile([128, 1], F16)
            nc.vector.memset(ones16[:], 1.0)

            # stats PSUM bank: colsums of x^2 on partition 0, scalar matmul
            # outputs parked at other partitions/columns of the same bank
            pstc = pstp.tile([128, 512], FP32)
            stat_ps = pstc[0:1, 0:512]

            # ---- downcast (ACT, S1 via accum) + squares (DVE) + colsums (PE)
            x8 = []
            sums = small.tile([128, 5], FP32)
            for j in range(DC // 2):
                t8 = x8_pool.tile([128, 2, S], F8, name=f"x8{j}", tag=f"x8{j}")
                if j < 3:
                    # whole-pair downcast in one ACT instr (S1 per pair)
                    nc.scalar.activation(t8[:], xt16[j][:],
                                         mybir.ActivationFunctionType.Copy,
                                         scale=X_SCL,
                                         accum_out=sums[:, j:j + 1])
                else:
                    nc.scalar.activation(t8[:, 0, :], xt16[j][:, 0, :],
                                         mybir.ActivationFunctionType.Copy,
                                         scale=X_SCL,
                                         accum_out=sums[:, 3:4])
                    nc.vector.tensor_scalar(t8[:, 1, :], xt16[j][:, 1, :],
                                            X_SCL, 0.0,
                                            mybir.AluOpType.mult,
                                            mybir.AluOpType.add,
                                            accum_out=sums[:, 4:5])
                sq = sq_pool.tile([128, 2, S], F16, name=f"sq{j}", tag="sq")
                nc.vector.tensor_tensor(sq[:], xt16[j][:], xt16[j][:],
                                        mybir.AluOpType.mult)
                for m in range(8):
                    nc.tensor.matmul(stat_ps[:],
                                     ones16[:],
                                     sq[:, m // 4, 512 * (m % 4):512 * (m % 4 + 1)],
                                     start=(j == 0 and m == 0),
                                     stop=(j == 3 and m == 7))
                x8.append(t8)

            # ---- stats scalar chain ----
            s1 = small.tile([128, 1], FP32)
            nc.vector.tensor_reduce(s1[:], sums[:], mybir.AxisListType.X,
                                    mybir.AluOpType.add)
            inv1_col = small.tile([128, 1], FP32)
            nc.vector.memset(inv1_col[:], inv1)
            ones_row = small.tile([1, 128], FP32)
            nc.vector.memset(ones_row[:], 1.0)

            nc.tensor.matmul(pstc[32:33, 0:1], inv1_col[:], s1[:],
                             start=True, stop=True)

            sc = small.tile([1, 8], FP32)
            mu, s2r, e2, nvar, std, rstd, rs, mrn = (sc[:, i:i + 1] for i in range(8))
            mr = small.tile([1, 1], FP32)
            eps_sb = small.tile([1, 1], FP32)
            nc.vector.memset(eps_sb[:], EPS)
            nc.scalar.copy(mu, pstc[32:33, 0:1])
            nc.vector.tensor_reduce(s2r, stat_ps[:], mybir.AxisListType.X,
                                    mybir.AluOpType.add)
            nc.vector.tensor_scalar_mul(e2, s2r, inv2)
            # nvar = mu^2 - e2 ; std = sqrt(-nvar + eps) ; rstd = 1/std
            nc.vector.scalar_tensor_tensor(nvar, mu, mu, e2,
                                           mybir.AluOpType.mult,
                                           mybir.AluOpType.subtract)
            nc.scalar.activation(std, nvar, mybir.ActivationFunctionType.Sqrt,
                                 bias=eps_sb[:], scale=-1.0)
            nc.vector.reciprocal(rstd, std)
            nc.vector.tensor_scalar_mul(rs, rstd, H_SCL * PSD_INV)
            nc.vector.tensor_tensor(mr, mu, rstd, mybir.AluOpType.mult)
            nc.vector.tensor_scalar_mul(mrn, mr, -H_SCL)

            nc.tensor.matmul(pstc[:, 2:4], ones_row[:], sc[:, 6:8],
                             start=True, stop=True)
            bc = small.tile([128, 2], FP32)
            nc.scalar.copy(bc[:], pstc[:, 2:4])

            bias_sb = small.tile([128, OC], FP32)
            nc.vector.scalar_tensor_tensor(
                bias_sb[:], pq_sb[:, OC:2 * OC], bc[:, 1:2], pq_sb[:, 0:OC],
                mybir.AluOpType.mult, mybir.AluOpType.add)

            # ---- down-proj (fp8 DoubleRow, 1024-wide moving) + ReLU ----
            for ot in range(OC):
                osz = O_SZ[ot]
                for sbpp in range(NSBP // 2):
                    ph = pmd.tile([128, 1024], FP32, name=f"ph{ot}_{sbpp}", tag="mmd")
                    for half in range(2):
                        sbp = 2 * sbpp + half
                        for kk in range(4):
                            nc.tensor.matmul(
                                ph[:osz, 512 * half:512 * (half + 1)],
                                wd_sb[:, 2 * kk:2 * kk + 2, 128 * ot:128 * ot + osz],
                                x8[kk][:, :, 512 * sbp:512 * (sbp + 1)],
                                start=(kk == 0), stop=(kk == 3), perf_mode=DR)
                    nc.scalar.activation(
                        ht[ot // 2][:osz, ot % 2, 1024 * sbpp:1024 * (sbpp + 1)],
                        ph[:osz, :],
                        mybir.ActivationFunctionType.Relu,
                        bias=bias_sb[:osz, ot:ot + 1], scale=bc[:osz, 0:1])

            # ---- up-proj (fp8 DoubleRow) + residual + store ----
            for sbp in range(NSBP):
                yo = yo_pool.tile([128, DC, 512], F16, name=f"yo{sbp}", tag="yo")
                for dt in range(DC):
                    pu = pmu.tile([128, 512], FP32, name=f"pu{dt}_{sbp}", tag="mmu")
                    for kk in range(2):
                        nc.tensor.matmul(
                            pu[:],
                            wu_sb[:, 2 * kk:2 * kk + 2, 128 * dt:128 * (dt + 1)],
                            ht[kk][:, :, 512 * sbp:512 * (sbp + 1)],
                            start=(kk == 0), stop=(kk == 1), perf_mode=DR)
                    xs = xt16[dt // 2][:, dt % 2, 512 * sbp:512 * (sbp + 1)]
                    if dt < 6:
                        nc.vector.scalar_tensor_tensor(
                            yo[:, dt, :], pu[:], PSU_INV, xs,
                            mybir.AluOpType.mult, mybir.AluOpType.add)
                    else:
                        nc.scalar.activation(yo[:, dt, :], pu[:],
                                             mybir.ActivationFunctionType.Copy,
                                             scale=PSU_INV)
                        nc.gpsimd.tensor_tensor(yo[:, dt, :], yo[:, dt, :], xs,
                                                mybir.AluOpType.add)
                nc.sync.dma_start(yT[:, sbp, 0:4, :], yo[:, 0:4, :])
                nc.sync.dma_start(yT[:, sbp, 4:8, :], yo[:, 4:8, :])

    nc.finalize()
    return nc


# ---------------------------------------------------------------------------
# Host-side orchestration
# ---------------------------------------------------------------------------

def prep_merge_inputs(alphas, W_down_all, W_up_all, W_ln_all, b_ln_all):
    a_in = np.ascontiguousarray(alphas.reshape(1, N)).astype(np.float32)
    wln = W_ln_all.reshape(N, DC, 128).transpose(0, 2, 1)
    bln = b_ln_all.reshape(N, DC, 128).transpose(0, 2, 1)
    ln_blk = np.concatenate([wln, bln], axis=2)             # [N,128,16]
    in_maps = []
    for k in range(NCORES):
        wd_k = W_down_all[:, WD_ROWS * k:WD_ROWS * (k + 1), :].reshape(N, 128, 400)
        wu_k = W_up_all[:, WU_ROWS * k:WU_ROWS * (k + 1), :]
        stack = np.concatenate([wd_k, wu_k, ln_blk], axis=2)
        # all adapters side-by-side in the free dim, fp8e4 at x64 scale
        stack = stack.transpose(1, 0, 2).reshape(128, N * MF)
        stack = _to_f8(stack * 64.0)
        in_maps.append({"stack": np.ascontiguousarray(stack).view(np.uint8),
                        "alphas": a_in})
    return in_maps


def _to_f8(a):
    return np.clip(a, -F8_MAX, F8_MAX).astype(NP_F8)


def assemble_merge(results):
    W_down = np.concatenate(
        [results[k]["out_m"][:, 0:400].astype(np.float32).reshape(WD_ROWS, D)
         for k in range(NCORES)], axis=0) / 64.0            # [BOT, D]
    W_up = np.concatenate(
        [results[k]["out_m"][:, 400:800].astype(np.float32)
         for k in range(NCORES)], axis=0) / 64.0            # [D, BOT]
    ln = results[0]["out_m"][:, 800:].astype(np.float32) / 64.0
    W_ln = ln[:, 0:DC].T.reshape(D)
    b_ln = ln[:, DC:2 * DC].T.reshape(D)

    wdT = W_down.T * (W_ln * W_SCL)[:, None]
    wd8 = _to_f8(wdT.reshape(DC, 128, BOT).transpose(1, 0, 2))

    wuT_pad = np.zeros((4 * 128, D), dtype=np.float32)
    wuT_pad[:BOT] = W_up.T * WU_SCL
    wu8 = _to_f8(wuT_pad.reshape(OC, 128, D).transpose(1, 0, 2))

    P = W_down @ b_ln
    Q = W_down @ W_ln
    pq = np.zeros((128, 2 * OC), dtype=np.float32)
    Pp = np.zeros(512, dtype=np.float32); Pp[:BOT] = H_SCL * P
    Qp = np.zeros(512, dtype=np.float32); Qp[:BOT] = Q
    pq[:, 0:OC] = Pp.reshape(OC, 128).T
    pq[:, OC:2 * OC] = Qp.reshape(OC, 128).T
    return (np.ascontiguousarray(wd8).view(np.uint8),
            np.ascontiguousarray(wu8).view(np.uint8),
            np.ascontiguousarray(pq))


def prep_main_inputs(x, wd8, wu8, pq):
    in_maps = []
    for k in range(NCORES):
        xt = x[k].T.reshape(DC, 128, S).transpose(1, 0, 2).astype(np.float16)
        in_maps.append({"xT16": np.ascontiguousarray(xt),
                        "wd8": wd8, "wu8": wu8, "pq": pq})
    return in_maps


def assemble_output(results):
    out = np.empty((B, S, D), dtype=np.float32)
    for k in range(NCORES):
        y = results[k]["yT"].astype(np.float32)   # [128, NSBP, DC, 512]
        out[k] = y.transpose(1, 3, 2, 0).reshape(S, D)
    return out


_NC_CACHE = {}


def _get_nc(which):
    if which not in _NC_CACHE:
        _NC_CACHE[which] = build_merge_nc() if which == "merge" else build_main_nc()
    return _NC_CACHE[which]


def run(inputs, trace=False, trace_cores=None):
    core_ids = list(range(NCORES))
    nc_a = _get_nc("merge")
    in_a = prep_merge_inputs(inputs["alphas"], inputs["W_down_all"],
                             inputs["W_up_all"], inputs["W_ln_all"],
                             inputs["b_ln_all"])
    res_a = run_bass_kernel_spmd(nc_a, in_a, core_ids=core_ids, trace=trace,
                                 trace_cores=trace_cores)
    wd8, wu8, pq = assemble_merge(res_a.results)

    nc_b = _get_nc("main")
    in_b = prep_main_inputs(inputs["x"], wd8, wu8, pq)
    res_b = run_bass_kernel_spmd(nc_b, in_b, core_ids=core_ids, trace=trace,
                                 trace_cores=trace_cores)
    out = assemble_output(res_b.results)
    return out, res_a, res_b


def kernel(**inputs):
    inputs = {k: np.asarray(v, dtype=np.float32) for k, v in inputs.items()}
    out, _, _ = run(inputs)
    return out
